# revision 34
# baseline (speedup 1.0000x reference)
"""MinLSTM cell for Trainium2 (Bass/Tile), data-parallel over batch on 8 cores.

Per core (one batch row):
  - xT [D,T] in SBUF (fp16); three projections W^T.T @ xT -> [H,T] as fp16
    matmuls (full PE rate, 1 cyc/col, ~1e-3 rel err) accumulating K=768
    into fp32 PSUM. fp16 (vs fp32r) hides LDWEIGHTS (2-elem XBUS) and
    halves x/w DMA bytes - the head is DMA-arrival-bound at ~150 GB/s
    per queue.
  - division-free gates: with Ef = e^{-zf}, Ei = e^{-zi} (ScalarE Exp
    straight from PSUM, bias fused), a = f/(f+i) = (1+Ei)/(2+Ef+Ei) and
    1/s2 = Exp(-Ln(s2)) - all ACT funcs in one LUT table. Gate tiles are
    fp16: 2x DVE/ScalarE throughput on the 16-bit elementwise ops.
  - h_t = a_t*h_{t-1} + b_t as one VectorE tensor_tensor_scan per
    [128,TC] tile (fp32 state/output), chained across T-chunks via the
    last column of the previous chunk's output.
  - output written as hT [H,T] fp32; host transposes back to [T,H].

DMA schedule: per-queue BW is only ~120-200 GB/s, so chunk-0 x and the
three weight matrices are striped across the sync+gpsimd queues in
need-order (x0, wf, wi, wh); the Activation queue carries no head DMAs
(a DMA_DIRECT2D occupies the engine ~650 ns, delaying the PSUM-draining
ACTs). Output DMAs alternate gpsimd/scalar to halve the tail drain.
"""

import sys

if "/opt/trn_rl_repo" not in sys.path:
    sys.path.insert(0, "/opt/trn_rl_repo")

import numpy as np

B, T, D, H = 8, 4096, 768, 768
TC = 512                    # time-chunk (one PSUM bank of fp32)
NT = T // TC                # 8 chunks
KD = D // 128               # 6 contraction tiles
MH = H // 128               # 6 hidden tiles

_state = {}


def _build():
    import concourse.mybir as mybir
    import concourse.tile as tile
    from concourse import bacc


    f32, f16 = mybir.dt.float32, mybir.dt.bfloat16
    A = mybir.AluOpType
    Act = mybir.ActivationFunctionType

    nc = bacc.Bacc("TRN2", target_bir_lowering=False, debug=False, num_devices=B)

    xT = nc.dram_tensor("xT", [D, T], f16, kind="ExternalInput")
    w_d = {p: nc.dram_tensor(f"w{p}", [D, H], f16, kind="ExternalInput") for p in "fih"}
    b_d = {p: nc.dram_tensor(f"b{p}", [128, MH], f32, kind="ExternalInput") for p in "fi"}
    h0_d = nc.dram_tensor("h0c", [128, MH], f32, kind="ExternalInput")
    hT = nc.dram_tensor("hT", [H, T], f16, kind="ExternalOutput")

    with tile.TileContext(nc) as tc:
        with (
            tc.tile_pool(name="wpool", bufs=1) as wpool,
            tc.tile_pool(name="cpool", bufs=1) as cpool,
            tc.tile_pool(name="xpool", bufs=2) as xpool,
            tc.tile_pool(name="psfi", bufs=5, space="PSUM") as psfi,
            tc.tile_pool(name="psh", bufs=3, space="PSUM") as psh,
            tc.tile_pool(name="wk", bufs=4) as wk,
            tc.tile_pool(name="hpool", bufs=3) as hpool,
        ):
            # Head DMAs striped across all three queues (per-queue BW is only
            # ~100-200 GB/s and nothing flows during the ~8us NEFF preamble,
            # so per-queue ORDER is everything). The chunk-0 f-GEMMs run
            # kd-major, so each queue sends wf[kd] immediately followed by
            # x0[kd] — the PE starts on the kd=0 sweep as soon as the first
            # two tiles land, while later kd tiles are still in flight.
            head_q = [nc.sync, nc.gpsimd, nc.scalar]
            w_sb = {p: [] for p in "fih"}
            b_sb = {}
            xs0 = [None] * KD

            def w_tile(p, kd):
                t = wpool.tile([128, H], f16, tag=f"w{p}{kd}", name=f"w{p}{kd}")
                head_q[kd % 3].dma_start(t[:], w_d[p][kd * 128:(kd + 1) * 128, :])
                w_sb[p].append(t)

            def b_tile(p):
                b_sb[p] = cpool.tile([128, MH], f32, tag=f"b{p}", name=f"bs{p}")
                nc.scalar.dma_start(b_sb[p][:], b_d[p][:])

            for kd in range(KD):
                w_tile("f", kd)
                # x0[kd] rides a DIFFERENT queue than wf[kd] so the first
                # sweep's two tiles transfer in parallel.
                xt = xpool.tile([128, TC], f16, tag=f"x{kd}", name=f"x0_{kd}")
                head_q[(kd + 1) % 3].dma_start(xt[:], xT[kd * 128:(kd + 1) * 128, 0:TC])
                xs0[kd] = xt
            b_tile("f")
            b_tile("i")
            for kd in range(KD):
                w_tile("i", kd)
            h0_sb = cpool.tile([128, MH], f32, tag="h0")
            nc.scalar.dma_start(h0_sb[:], h0_d[:])
            for kd in range(KD):
                w_tile("h", kd)

            # Full chunks, except the last is split in two so the serial
            # consumer chain after the very last matmul runs at half width.
            segs = [(c * TC, TC) for c in range(NT - 1)]
            segs += [((NT - 1) * TC, TC // 2), ((NT - 1) * TC + TC // 2, TC // 2)]

            prev_h = [None] * MH
            prev_len = 0
            for c, (off, L) in enumerate(segs):
                if c == 0:
                    xs = xs0
                else:
                    xs = []
                    for kd in range(KD):
                        xt = xpool.tile([128, TC], f16, tag=f"x{kd}", name=f"x{c}_{kd}")
                        nc.sync.dma_start(xt[:, 0:L], xT[kd * 128:(kd + 1) * 128, off:off + L])
                        xs.append(xt)

                def alloc_ps(p, j, pool=None):
                    # ps_h lives until the DVE bt op (late); f/i banks free at
                    # their Exp ACTs. Separate pools so the FIFO rotation of
                    # the short-lived f/i banks never waits behind an h bank.
                    if pool is None:
                        pool = psh if p == "h" else psfi
                    return pool.tile([128, TC], f32, tag="ps", name=f"ps{c}_{j}_{p}")

                def mm(pt, p, j, kd):
                    nc.tensor.matmul(
                        pt[:, 0:L],
                        w_sb[p][kd][:, j * 128:(j + 1) * 128],
                        xs[kd][:, 0:L],
                        start=(kd == 0),
                        stop=(kd == KD - 1),
                    )

                def emit_group(p, j, ps):
                    pt = alloc_ps(p, j)
                    for kd in range(KD):
                        mm(pt, p, j, kd)
                    ps[p] = pt

                # Chunk 0: p-major so the PE streams all f-groups while the
                # wi/wh weight DMAs are still in flight; the f-groups run
                # kd-MAJOR across all six j so the PE consumes each wf[kd]
                # tile as it lands instead of stalling for the whole matrix
                # (head DMA arrival is the bottleneck). Needs 6 concurrent
                # banks: borrow two from the h pool (freed by ef ACTs long
                # before the first h-group).
                ps_by_j = [dict() for _ in range(MH)]
                if c == 0:
                    for j in range(MH):
                        ps_by_j[j]["f"] = alloc_ps("f", j, pool=psfi if j < 4 else psh)
                    for kd in range(KD):
                        for j in range(MH):
                            mm(ps_by_j[j]["f"], "f", j, kd)
                    for p in "ih":
                        for j in range(MH):
                            emit_group(p, j, ps_by_j[j])
                for j in range(MH):
                    ps = ps_by_j[j]
                    if c != 0:
                        for p in "fih":
                            emit_group(p, j, ps)
                    # Division-free gates via Exp/Ln (single ACT table):
                    # with Ef = e^{-zf}, Ei = e^{-zi}:  a = f/(f+i) =
                    # (1+Ei)/(2+Ef+Ei) and 1/s2 = Exp(-Ln(s2)). The i-gate
                    # is 1-a by construction (exactly, including LUT error):
                    #   b' = (a - 1) * zh,  h = a*h_prev - b'
                    # so the ut tile, its STT, and the zh bias-add all
                    # disappear (h_b is folded in as a constant output shift
                    # applied on the host; exact because the gates sum to 1).
                    ef = wk.tile([128, TC], f32, tag="ef")
                    nc.scalar.activation(ef[:, 0:L], ps["f"][:, 0:L], Act.Exp, bias=b_sb["f"][:, j:j + 1], scale=-1.0)
                    ei = wk.tile([128, TC], f32, tag="ei")
                    nc.scalar.activation(ei[:, 0:L], ps["i"][:, 0:L], Act.Exp, bias=b_sb["i"][:, j:j + 1], scale=-1.0)
                    s2 = wk.tile([128, TC], f32, tag="s2")
                    nc.vector.scalar_tensor_tensor(s2[:, 0:L], ef[:, 0:L], 2.0, ei[:, 0:L], A.add, A.add)
                    ln2 = wk.tile([128, TC], f32, tag="ln2")
                    nc.scalar.activation(ln2[:, 0:L], s2[:, 0:L], Act.Ln, bias=0.0, scale=1.0)
                    rt = wk.tile([128, TC], f32, tag="rt")
                    nc.scalar.activation(rt[:, 0:L], ln2[:, 0:L], Act.Exp, bias=0.0, scale=-1.0)
                    # ht: PSUM -> SBUF copy on ScalarE. Cheaper-looking is to
                    # read ps_h directly in the bt STT, but that holds the h
                    # bank hostage to the DVE queue (which trails behind the
                    # scan chain) and stalls PE group starts on bank frees.
                    # On the LAST chunk no matmuls follow, so read the PSUM
                    # directly and keep ScalarE out of the final drain.
                    last = c >= len(segs) - 2
                    if not last:
                        ht = wk.tile([128, TC], f32, tag="ht")
                        nc.scalar.activation(ht[:, 0:L], ps["h"][:, 0:L], Act.Identity, bias=0.0, scale=1.0)
                        h_src = ht
                    else:
                        h_src = ps["h"]
                    at = wk.tile([128, TC], f32, tag="a")
                    nc.vector.scalar_tensor_tensor(at[:, 0:L], ei[:, 0:L], 1.0, rt[:, 0:L], A.add, A.mult)
                    bt = wk.tile([128, TC], f32, tag="b")
                    nc.vector.scalar_tensor_tensor(bt[:, 0:L], at[:, 0:L], 1.0, h_src[:, 0:L], A.subtract, A.mult)
                    # h tiles are bf16: halves the 12.6 MB output traffic
                    # (the kernel tail was output-DMA wire drain). The scan
                    # state stays fp32 internally; the once-per-chunk bf16
                    # rounding of the chaining column is negligible.
                    hh = hpool.tile([128, TC], f16, tag=f"h{j}")
                    init = h0_sb[:, j:j + 1] if c == 0 else prev_h[j][:, prev_len - 1:prev_len]
                    nc.vector.tensor_tensor_scan(hh[:, 0:L], at[:, 0:L], bt[:, 0:L], init, op0=A.mult, op1=A.subtract)
                    prev_h[j] = hh
                    # Output stays off the Activation queue mid-kernel (a DMA
                    # issue there delays the PSUM-draining ACTs -> PE stalls);
                    # on the last chunk sync is idle, so alternate with it to
                    # halve the final wire drain.
                    out_q = nc.gpsimd if (c < NT - 1 or j % 2 == 0) else nc.sync
                    out_q.dma_start(hT[j * 128:(j + 1) * 128, off:off + L], hh[:, 0:L])
                prev_len = L

    # All ACT funcs used (Exp, Ln, Identity) live in the single table
    # "natural_log_exp_and_others", but the table-load pass picks the FIRST
    # table containing each func. Empty out every other table (names and
    # positions preserved so runtime table ids stay valid) so a single
    # table load is emitted.
    import concourse.bacc as bacc_mod

    orig_tables = bacc_mod.get_activation_tables

    def _single_table(arch):
        tabs = orig_tables(arch)
        keep = "natural_log_exp_and_others"
        return {k: (v if k == keep else set()) for k, v in tabs.items()}

    bacc_mod.get_activation_tables = _single_table
    try:
        nc.compile()
    finally:
        bacc_mod.get_activation_tables = orig_tables
    return nc


def _get_nc():
    if "nc" not in _state:
        _state["nc"] = _build()
    return _state["nc"]


def _prep_inputs(x, h0, f_w, f_b, i_w, i_b, h_w, h_b):
    x = np.asarray(x, dtype=np.float32)
    h0 = np.asarray(h0, dtype=np.float32)
    import ml_dtypes

    bf16 = ml_dtypes.bfloat16
    xT = np.ascontiguousarray(x.transpose(0, 2, 1).astype(bf16))  # [B, D, T]
    hb = np.asarray(h_b, dtype=np.float32)
    shared = {}
    for p, w, bias in (("f", f_w, f_b), ("i", i_w, i_b), ("h", h_w, None)):
        w = np.asarray(w, dtype=np.float32)
        shared[f"w{p}"] = np.ascontiguousarray(w.T.astype(bf16))  # [D, H]
        if bias is not None:
            # f/i biases negated: kernel computes Exp(-pre + bias_ap)
            bias = -np.asarray(bias, dtype=np.float32)
            shared[f"b{p}"] = np.ascontiguousarray(bias.reshape(MH, 128).T)  # [128, MH]
    in_maps = []
    for b in range(B):
        m = dict(shared)
        m["xT"] = xT[b]
        # h_b is applied as a constant output shift (see kernel comment):
        # the device recurrence runs on h' = h - h_b.
        m["h0c"] = np.ascontiguousarray((h0[b, 0] - hb).reshape(MH, 128).T)
        in_maps.append(m)
    return in_maps


def kernel(x, h0, f_w, f_b, i_w, i_b, h_w, h_b, _trace=False):
    from concourse.bass_utils import run_bass_kernel_spmd

    nc = _get_nc()
    in_maps = _prep_inputs(x, h0, f_w, f_b, i_w, i_b, h_w, h_b)
    res = run_bass_kernel_spmd(nc, in_maps, core_ids=list(range(B)), trace=_trace)
    hb = np.asarray(h_b, dtype=np.float32)
    out = np.empty((B, T, H), dtype=np.float32)
    for b in range(B):
        out[b] = res.results[b]["hT"].astype(np.float32).T + hb
    if _trace:
        _state["last_results"] = res
    return out


# revision 35
# speedup vs baseline: 1.0078x; 1.0078x over previous
"""MinLSTM cell for Trainium2 (Bass/Tile), data-parallel over batch on 8 cores.

Per core (one batch row):
  - xT [D,T] in SBUF (fp16); three projections W^T.T @ xT -> [H,T] as fp16
    matmuls (full PE rate, 1 cyc/col, ~1e-3 rel err) accumulating K=768
    into fp32 PSUM. fp16 (vs fp32r) hides LDWEIGHTS (2-elem XBUS) and
    halves x/w DMA bytes - the head is DMA-arrival-bound at ~150 GB/s
    per queue.
  - division-free gates: with Ef = e^{-zf}, Ei = e^{-zi} (ScalarE Exp
    straight from PSUM, bias fused), a = f/(f+i) = (1+Ei)/(2+Ef+Ei) and
    1/s2 = Exp(-Ln(s2)) - all ACT funcs in one LUT table. Gate tiles are
    fp16: 2x DVE/ScalarE throughput on the 16-bit elementwise ops.
  - h_t = a_t*h_{t-1} + b_t as one VectorE tensor_tensor_scan per
    [128,TC] tile (fp32 state/output), chained across T-chunks via the
    last column of the previous chunk's output.
  - output written as hT [H,T] fp32; host transposes back to [T,H].

DMA schedule: per-queue BW is only ~120-200 GB/s, so chunk-0 x and the
three weight matrices are striped across the sync+gpsimd queues in
need-order (x0, wf, wi, wh); the Activation queue carries no head DMAs
(a DMA_DIRECT2D occupies the engine ~650 ns, delaying the PSUM-draining
ACTs). Output DMAs alternate gpsimd/scalar to halve the tail drain.
"""

import sys

if "/opt/trn_rl_repo" not in sys.path:
    sys.path.insert(0, "/opt/trn_rl_repo")

import numpy as np

B, T, D, H = 8, 4096, 768, 768
TC = 512                    # time-chunk (one PSUM bank of fp32)
NT = T // TC                # 8 chunks
KD = D // 128               # 6 contraction tiles
MH = H // 128               # 6 hidden tiles

_state = {}


def _build():
    import concourse.mybir as mybir
    import concourse.tile as tile
    from concourse import bacc


    f32, f16 = mybir.dt.float32, mybir.dt.bfloat16
    A = mybir.AluOpType
    Act = mybir.ActivationFunctionType

    nc = bacc.Bacc("TRN2", target_bir_lowering=False, debug=False, num_devices=B)

    xT = nc.dram_tensor("xT", [D, T], f16, kind="ExternalInput")
    w_d = {p: nc.dram_tensor(f"w{p}", [D, H], f16, kind="ExternalInput") for p in "fih"}
    b_d = {p: nc.dram_tensor(f"b{p}", [128, MH], f32, kind="ExternalInput") for p in "fi"}
    h0_d = nc.dram_tensor("h0c", [128, MH], f32, kind="ExternalInput")
    hT = nc.dram_tensor("hT", [H, T], f16, kind="ExternalOutput")

    with tile.TileContext(nc) as tc:
        with (
            tc.tile_pool(name="wpool", bufs=1) as wpool,
            tc.tile_pool(name="cpool", bufs=1) as cpool,
            tc.tile_pool(name="xpool", bufs=2) as xpool,
            tc.tile_pool(name="psfi", bufs=5, space="PSUM") as psfi,
            tc.tile_pool(name="psh", bufs=3, space="PSUM") as psh,
            tc.tile_pool(name="wk", bufs=4) as wk,
            tc.tile_pool(name="hpool", bufs=3) as hpool,
        ):
            # Head DMAs striped across all three queues (per-queue BW is only
            # ~100-200 GB/s and nothing flows during the ~8us NEFF preamble,
            # so per-queue ORDER is everything). The chunk-0 f-GEMMs run
            # kd-major, so each queue sends wf[kd] immediately followed by
            # x0[kd] — the PE starts on the kd=0 sweep as soon as the first
            # two tiles land, while later kd tiles are still in flight.
            head_q = [nc.sync, nc.gpsimd, nc.scalar]
            w_sb = {p: [] for p in "fih"}
            b_sb = {}
            xs0 = [None] * KD

            def w_tile(p, kd):
                t = wpool.tile([128, H], f16, tag=f"w{p}{kd}", name=f"w{p}{kd}")
                head_q[kd % 3].dma_start(t[:], w_d[p][kd * 128:(kd + 1) * 128, :])
                w_sb[p].append(t)

            def b_tile(p):
                b_sb[p] = cpool.tile([128, MH], f32, tag=f"b{p}", name=f"bs{p}")
                nc.scalar.dma_start(b_sb[p][:], b_d[p][:])

            for kd in range(KD):
                w_tile("f", kd)
                xt = xpool.tile([128, TC], f16, tag=f"x{kd}", name=f"x0_{kd}")
                head_q[kd % 3].dma_start(xt[:], xT[kd * 128:(kd + 1) * 128, 0:TC])
                xs0[kd] = xt
            b_tile("f")
            b_tile("i")
            for kd in range(KD):
                w_tile("i", kd)
            h0_sb = cpool.tile([128, MH], f32, tag="h0")
            nc.scalar.dma_start(h0_sb[:], h0_d[:])
            for kd in range(KD):
                w_tile("h", kd)

            # Full chunks, except the last is split in two so the serial
            # consumer chain after the very last matmul runs at half width.
            segs = [(c * TC, TC) for c in range(NT - 1)]
            segs += [((NT - 1) * TC, TC // 2), ((NT - 1) * TC + TC // 2, TC // 2)]

            prev_h = [None] * MH
            prev_len = 0
            for c, (off, L) in enumerate(segs):
                if c == 0:
                    xs = xs0
                else:
                    xs = []
                    for kd in range(KD):
                        xt = xpool.tile([128, TC], f16, tag=f"x{kd}", name=f"x{c}_{kd}")
                        nc.sync.dma_start(xt[:, 0:L], xT[kd * 128:(kd + 1) * 128, off:off + L])
                        xs.append(xt)

                def alloc_ps(p, j, pool=None):
                    # ps_h lives until the DVE bt op (late); f/i banks free at
                    # their Exp ACTs. Separate pools so the FIFO rotation of
                    # the short-lived f/i banks never waits behind an h bank.
                    if pool is None:
                        pool = psh if p == "h" else psfi
                    return pool.tile([128, TC], f32, tag="ps", name=f"ps{c}_{j}_{p}")

                def mm(pt, p, j, kd):
                    nc.tensor.matmul(
                        pt[:, 0:L],
                        w_sb[p][kd][:, j * 128:(j + 1) * 128],
                        xs[kd][:, 0:L],
                        start=(kd == 0),
                        stop=(kd == KD - 1),
                    )

                def emit_group(p, j, ps):
                    pt = alloc_ps(p, j)
                    for kd in range(KD):
                        mm(pt, p, j, kd)
                    ps[p] = pt

                # Chunk 0: p-major so the PE streams all f-groups while the
                # wi/wh weight DMAs are still in flight; the f-groups run
                # kd-MAJOR across all six j so the PE consumes each wf[kd]
                # tile as it lands instead of stalling for the whole matrix
                # (head DMA arrival is the bottleneck). Needs 6 concurrent
                # banks: borrow two from the h pool (freed by ef ACTs long
                # before the first h-group).
                ps_by_j = [dict() for _ in range(MH)]
                if c == 0:
                    for j in range(MH):
                        ps_by_j[j]["f"] = alloc_ps("f", j, pool=psfi if j < 4 else psh)
                    for kd in range(KD):
                        for j in range(MH):
                            mm(ps_by_j[j]["f"], "f", j, kd)
                    for p in "ih":
                        for j in range(MH):
                            emit_group(p, j, ps_by_j[j])
                for j in range(MH):
                    ps = ps_by_j[j]
                    if c != 0:
                        for p in "fih":
                            emit_group(p, j, ps)
                    # Division-free gates via Exp/Ln (single ACT table):
                    # with Ef = e^{-zf}, Ei = e^{-zi}:  a = f/(f+i) =
                    # (1+Ei)/(2+Ef+Ei) and 1/s2 = Exp(-Ln(s2)). The i-gate
                    # is 1-a by construction (exactly, including LUT error):
                    #   b' = (a - 1) * zh,  h = a*h_prev - b'
                    # so the ut tile, its STT, and the zh bias-add all
                    # disappear (h_b is folded in as a constant output shift
                    # applied on the host; exact because the gates sum to 1).
                    ef = wk.tile([128, TC], f32, tag="ef")
                    nc.scalar.activation(ef[:, 0:L], ps["f"][:, 0:L], Act.Exp, bias=b_sb["f"][:, j:j + 1], scale=-1.0)
                    ei = wk.tile([128, TC], f32, tag="ei")
                    nc.scalar.activation(ei[:, 0:L], ps["i"][:, 0:L], Act.Exp, bias=b_sb["i"][:, j:j + 1], scale=-1.0)
                    s2 = wk.tile([128, TC], f32, tag="s2")
                    nc.vector.scalar_tensor_tensor(s2[:, 0:L], ef[:, 0:L], 2.0, ei[:, 0:L], A.add, A.add)
                    ln2 = wk.tile([128, TC], f32, tag="ln2")
                    nc.scalar.activation(ln2[:, 0:L], s2[:, 0:L], Act.Ln, bias=0.0, scale=1.0)
                    rt = wk.tile([128, TC], f32, tag="rt")
                    nc.scalar.activation(rt[:, 0:L], ln2[:, 0:L], Act.Exp, bias=0.0, scale=-1.0)
                    # ht: PSUM -> SBUF copy on ScalarE. Cheaper-looking is to
                    # read ps_h directly in the bt STT, but that holds the h
                    # bank hostage to the DVE queue (which trails behind the
                    # scan chain) and stalls PE group starts on bank frees.
                    # On the LAST chunk no matmuls follow, so read the PSUM
                    # directly and keep ScalarE out of the final drain.
                    last = c >= len(segs) - 2
                    if not last:
                        ht = wk.tile([128, TC], f32, tag="ht")
                        nc.scalar.activation(ht[:, 0:L], ps["h"][:, 0:L], Act.Identity, bias=0.0, scale=1.0)
                        h_src = ht
                    else:
                        h_src = ps["h"]
                    at = wk.tile([128, TC], f32, tag="a")
                    nc.vector.scalar_tensor_tensor(at[:, 0:L], ei[:, 0:L], 1.0, rt[:, 0:L], A.add, A.mult)
                    bt = wk.tile([128, TC], f32, tag="b")
                    nc.vector.scalar_tensor_tensor(bt[:, 0:L], at[:, 0:L], 1.0, h_src[:, 0:L], A.subtract, A.mult)
                    # h tiles are bf16: halves the 12.6 MB output traffic
                    # (the kernel tail was output-DMA wire drain). The scan
                    # state stays fp32 internally; the once-per-chunk bf16
                    # rounding of the chaining column is negligible.
                    hh = hpool.tile([128, TC], f16, tag=f"h{j}")
                    init = h0_sb[:, j:j + 1] if c == 0 else prev_h[j][:, prev_len - 1:prev_len]
                    nc.vector.tensor_tensor_scan(hh[:, 0:L], at[:, 0:L], bt[:, 0:L], init, op0=A.mult, op1=A.subtract)
                    prev_h[j] = hh
                    # Output stays off the Activation queue mid-kernel (a DMA
                    # issue there delays the PSUM-draining ACTs -> PE stalls);
                    # on the last chunk sync is idle, so alternate with it to
                    # halve the final wire drain.
                    out_q = nc.gpsimd if (c < NT - 1 or j % 2 == 0) else nc.sync
                    out_q.dma_start(hT[j * 128:(j + 1) * 128, off:off + L], hh[:, 0:L])
                prev_len = L

    # All ACT funcs used (Exp, Ln, Identity) live in the single table
    # "natural_log_exp_and_others", but the table-load pass picks the FIRST
    # table containing each func. Empty out every other table (names and
    # positions preserved so runtime table ids stay valid) so a single
    # table load is emitted.
    import concourse.bacc as bacc_mod

    orig_tables = bacc_mod.get_activation_tables

    def _single_table(arch):
        tabs = orig_tables(arch)
        keep = "natural_log_exp_and_others"
        return {k: (v if k == keep else set()) for k, v in tabs.items()}

    bacc_mod.get_activation_tables = _single_table
    try:
        nc.compile()
    finally:
        bacc_mod.get_activation_tables = orig_tables
    return nc


def _get_nc():
    if "nc" not in _state:
        _state["nc"] = _build()
    return _state["nc"]


def _prep_inputs(x, h0, f_w, f_b, i_w, i_b, h_w, h_b):
    x = np.asarray(x, dtype=np.float32)
    h0 = np.asarray(h0, dtype=np.float32)
    import ml_dtypes

    bf16 = ml_dtypes.bfloat16
    xT = np.ascontiguousarray(x.transpose(0, 2, 1).astype(bf16))  # [B, D, T]
    hb = np.asarray(h_b, dtype=np.float32)
    shared = {}
    for p, w, bias in (("f", f_w, f_b), ("i", i_w, i_b), ("h", h_w, None)):
        w = np.asarray(w, dtype=np.float32)
        shared[f"w{p}"] = np.ascontiguousarray(w.T.astype(bf16))  # [D, H]
        if bias is not None:
            # f/i biases negated: kernel computes Exp(-pre + bias_ap)
            bias = -np.asarray(bias, dtype=np.float32)
            shared[f"b{p}"] = np.ascontiguousarray(bias.reshape(MH, 128).T)  # [128, MH]
    in_maps = []
    for b in range(B):
        m = dict(shared)
        m["xT"] = xT[b]
        # h_b is applied as a constant output shift (see kernel comment):
        # the device recurrence runs on h' = h - h_b.
        m["h0c"] = np.ascontiguousarray((h0[b, 0] - hb).reshape(MH, 128).T)
        in_maps.append(m)
    return in_maps


def kernel(x, h0, f_w, f_b, i_w, i_b, h_w, h_b, _trace=False):
    from concourse.bass_utils import run_bass_kernel_spmd

    nc = _get_nc()
    in_maps = _prep_inputs(x, h0, f_w, f_b, i_w, i_b, h_w, h_b)
    res = run_bass_kernel_spmd(nc, in_maps, core_ids=list(range(B)), trace=_trace)
    hb = np.asarray(h_b, dtype=np.float32)
    out = np.empty((B, T, H), dtype=np.float32)
    for b in range(B):
        out[b] = res.results[b]["hT"].astype(np.float32).T + hb
    if _trace:
        _state["last_results"] = res
    return out


# revision 36
# speedup vs baseline: 1.0096x; 1.0018x over previous
"""MinLSTM cell for Trainium2 (Bass/Tile), data-parallel over batch on 8 cores.

Per core (one batch row):
  - xT [D,T] in SBUF (fp16); three projections W^T.T @ xT -> [H,T] as fp16
    matmuls (full PE rate, 1 cyc/col, ~1e-3 rel err) accumulating K=768
    into fp32 PSUM. fp16 (vs fp32r) hides LDWEIGHTS (2-elem XBUS) and
    halves x/w DMA bytes - the head is DMA-arrival-bound at ~150 GB/s
    per queue.
  - division-free gates: with Ef = e^{-zf}, Ei = e^{-zi} (ScalarE Exp
    straight from PSUM, bias fused), a = f/(f+i) = (1+Ei)/(2+Ef+Ei) and
    1/s2 = Exp(-Ln(s2)) - all ACT funcs in one LUT table. Gate tiles are
    fp16: 2x DVE/ScalarE throughput on the 16-bit elementwise ops.
  - h_t = a_t*h_{t-1} + b_t as one VectorE tensor_tensor_scan per
    [128,TC] tile (fp32 state/output), chained across T-chunks via the
    last column of the previous chunk's output.
  - output written as hT [H,T] fp32; host transposes back to [T,H].

DMA schedule: per-queue BW is only ~120-200 GB/s, so chunk-0 x and the
three weight matrices are striped across the sync+gpsimd queues in
need-order (x0, wf, wi, wh); the Activation queue carries no head DMAs
(a DMA_DIRECT2D occupies the engine ~650 ns, delaying the PSUM-draining
ACTs). Output DMAs alternate gpsimd/scalar to halve the tail drain.
"""

import sys

if "/opt/trn_rl_repo" not in sys.path:
    sys.path.insert(0, "/opt/trn_rl_repo")

import numpy as np

B, T, D, H = 8, 4096, 768, 768
TC = 512                    # time-chunk (one PSUM bank of fp32)
NT = T // TC                # 8 chunks
KD = D // 128               # 6 contraction tiles
MH = H // 128               # 6 hidden tiles

_state = {}


def _build():
    import concourse.mybir as mybir
    import concourse.tile as tile
    from concourse import bacc


    f32, f16 = mybir.dt.float32, mybir.dt.bfloat16
    A = mybir.AluOpType
    Act = mybir.ActivationFunctionType

    nc = bacc.Bacc("TRN2", target_bir_lowering=False, debug=False, num_devices=B)

    xT = nc.dram_tensor("xT", [D, T], f16, kind="ExternalInput")
    w_d = {p: nc.dram_tensor(f"w{p}", [D, H], f16, kind="ExternalInput") for p in "fih"}
    b_d = {p: nc.dram_tensor(f"b{p}", [128, MH], f32, kind="ExternalInput") for p in "fi"}
    h0_d = nc.dram_tensor("h0c", [128, MH], f32, kind="ExternalInput")
    hT = nc.dram_tensor("hT", [H, T], f16, kind="ExternalOutput")

    with tile.TileContext(nc) as tc:
        with (
            tc.tile_pool(name="wpool", bufs=1) as wpool,
            tc.tile_pool(name="cpool", bufs=1) as cpool,
            tc.tile_pool(name="xpool", bufs=2) as xpool,
            tc.tile_pool(name="psfi", bufs=6, space="PSUM") as psfi,
            tc.tile_pool(name="psh", bufs=2, space="PSUM") as psh,
            tc.tile_pool(name="wk", bufs=4) as wk,
            tc.tile_pool(name="hpool", bufs=3) as hpool,
        ):
            # Head DMAs striped across all three queues (per-queue BW is only
            # ~100-200 GB/s and nothing flows during the ~8us NEFF preamble,
            # so per-queue ORDER is everything). The chunk-0 f-GEMMs run
            # kd-major, so each queue sends wf[kd] immediately followed by
            # x0[kd] — the PE starts on the kd=0 sweep as soon as the first
            # two tiles land, while later kd tiles are still in flight.
            head_q = [nc.sync, nc.gpsimd, nc.scalar]
            w_sb = {p: [] for p in "fih"}
            b_sb = {}
            xs0 = [None] * KD

            def w_tile(p, kd):
                t = wpool.tile([128, H], f16, tag=f"w{p}{kd}", name=f"w{p}{kd}")
                head_q[kd % 3].dma_start(t[:], w_d[p][kd * 128:(kd + 1) * 128, :])
                w_sb[p].append(t)

            def b_tile(p):
                b_sb[p] = cpool.tile([128, MH], f32, tag=f"b{p}", name=f"bs{p}")
                nc.scalar.dma_start(b_sb[p][:], b_d[p][:])

            for kd in range(KD):
                w_tile("f", kd)
                xt = xpool.tile([128, TC], f16, tag=f"x{kd}", name=f"x0_{kd}")
                head_q[kd % 3].dma_start(xt[:], xT[kd * 128:(kd + 1) * 128, 0:TC])
                xs0[kd] = xt
            b_tile("f")
            b_tile("i")
            for kd in range(KD):
                w_tile("i", kd)
            h0_sb = cpool.tile([128, MH], f32, tag="h0")
            nc.scalar.dma_start(h0_sb[:], h0_d[:])
            for kd in range(KD):
                w_tile("h", kd)

            # Full chunks, except the last is split in two so the serial
            # consumer chain after the very last matmul runs at half width.
            segs = [(c * TC, TC) for c in range(NT - 1)]
            segs += [((NT - 1) * TC, TC // 2), ((NT - 1) * TC + TC // 2, TC // 2)]

            prev_h = [None] * MH
            prev_len = 0
            for c, (off, L) in enumerate(segs):
                if c == 0:
                    xs = xs0
                else:
                    xs = []
                    for kd in range(KD):
                        xt = xpool.tile([128, TC], f16, tag=f"x{kd}", name=f"x{c}_{kd}")
                        nc.sync.dma_start(xt[:, 0:L], xT[kd * 128:(kd + 1) * 128, off:off + L])
                        xs.append(xt)

                def alloc_ps(p, j, pool=None):
                    # ps_h lives until the DVE bt op (late); f/i banks free at
                    # their Exp ACTs. Separate pools so the FIFO rotation of
                    # the short-lived f/i banks never waits behind an h bank.
                    if pool is None:
                        pool = psh if p == "h" else psfi
                    return pool.tile([128, TC], f32, tag="ps", name=f"ps{c}_{j}_{p}")

                def mm(pt, p, j, kd):
                    nc.tensor.matmul(
                        pt[:, 0:L],
                        w_sb[p][kd][:, j * 128:(j + 1) * 128],
                        xs[kd][:, 0:L],
                        start=(kd == 0),
                        stop=(kd == KD - 1),
                    )

                def emit_group(p, j, ps):
                    pt = alloc_ps(p, j)
                    for kd in range(KD):
                        mm(pt, p, j, kd)
                    ps[p] = pt

                # Chunk 0: p-major so the PE streams all f-groups while the
                # wi/wh weight DMAs are still in flight; the f-groups run
                # kd-MAJOR across all six j so the PE consumes each wf[kd]
                # tile as it lands instead of stalling for the whole matrix
                # (head DMA arrival is the bottleneck). Needs 6 concurrent
                # banks: borrow two from the h pool (freed by ef ACTs long
                # before the first h-group).
                ps_by_j = [dict() for _ in range(MH)]
                if c == 0:
                    for j in range(MH):
                        ps_by_j[j]["f"] = alloc_ps("f", j, pool=psfi if j < 4 else psh)
                    for kd in range(KD):
                        for j in range(MH):
                            mm(ps_by_j[j]["f"], "f", j, kd)
                    for p in "ih":
                        for j in range(MH):
                            emit_group(p, j, ps_by_j[j])
                for j in range(MH):
                    ps = ps_by_j[j]
                    if c != 0:
                        for p in "fih":
                            emit_group(p, j, ps)
                    # Division-free gates via Exp/Ln (single ACT table):
                    # with Ef = e^{-zf}, Ei = e^{-zi}:  a = f/(f+i) =
                    # (1+Ei)/(2+Ef+Ei) and 1/s2 = Exp(-Ln(s2)). The i-gate
                    # is 1-a by construction (exactly, including LUT error):
                    #   b' = (a - 1) * zh,  h = a*h_prev - b'
                    # so the ut tile, its STT, and the zh bias-add all
                    # disappear (h_b is folded in as a constant output shift
                    # applied on the host; exact because the gates sum to 1).
                    ef = wk.tile([128, TC], f32, tag="ef")
                    nc.scalar.activation(ef[:, 0:L], ps["f"][:, 0:L], Act.Exp, bias=b_sb["f"][:, j:j + 1], scale=-1.0)
                    ei = wk.tile([128, TC], f32, tag="ei")
                    nc.scalar.activation(ei[:, 0:L], ps["i"][:, 0:L], Act.Exp, bias=b_sb["i"][:, j:j + 1], scale=-1.0)
                    s2 = wk.tile([128, TC], f32, tag="s2")
                    nc.vector.scalar_tensor_tensor(s2[:, 0:L], ef[:, 0:L], 2.0, ei[:, 0:L], A.add, A.add)
                    ln2 = wk.tile([128, TC], f32, tag="ln2")
                    nc.scalar.activation(ln2[:, 0:L], s2[:, 0:L], Act.Ln, bias=0.0, scale=1.0)
                    rt = wk.tile([128, TC], f32, tag="rt")
                    nc.scalar.activation(rt[:, 0:L], ln2[:, 0:L], Act.Exp, bias=0.0, scale=-1.0)
                    # ht: PSUM -> SBUF copy on ScalarE. Cheaper-looking is to
                    # read ps_h directly in the bt STT, but that holds the h
                    # bank hostage to the DVE queue (which trails behind the
                    # scan chain) and stalls PE group starts on bank frees.
                    # On the LAST chunk no matmuls follow, so read the PSUM
                    # directly and keep ScalarE out of the final drain.
                    last = c >= len(segs) - 2
                    if not last:
                        ht = wk.tile([128, TC], f32, tag="ht")
                        nc.scalar.activation(ht[:, 0:L], ps["h"][:, 0:L], Act.Identity, bias=0.0, scale=1.0)
                        h_src = ht
                    else:
                        h_src = ps["h"]
                    at = wk.tile([128, TC], f32, tag="a")
                    nc.vector.scalar_tensor_tensor(at[:, 0:L], ei[:, 0:L], 1.0, rt[:, 0:L], A.add, A.mult)
                    bt = wk.tile([128, TC], f32, tag="b")
                    nc.vector.scalar_tensor_tensor(bt[:, 0:L], at[:, 0:L], 1.0, h_src[:, 0:L], A.subtract, A.mult)
                    # h tiles are bf16: halves the 12.6 MB output traffic
                    # (the kernel tail was output-DMA wire drain). The scan
                    # state stays fp32 internally; the once-per-chunk bf16
                    # rounding of the chaining column is negligible.
                    hh = hpool.tile([128, TC], f16, tag=f"h{j}")
                    init = h0_sb[:, j:j + 1] if c == 0 else prev_h[j][:, prev_len - 1:prev_len]
                    nc.vector.tensor_tensor_scan(hh[:, 0:L], at[:, 0:L], bt[:, 0:L], init, op0=A.mult, op1=A.subtract)
                    prev_h[j] = hh
                    # Output stays off the Activation queue mid-kernel (a DMA
                    # issue there delays the PSUM-draining ACTs -> PE stalls);
                    # on the last chunk sync is idle, so alternate with it to
                    # halve the final wire drain.
                    out_q = nc.gpsimd if (c < NT - 1 or j % 2 == 0) else nc.sync
                    out_q.dma_start(hT[j * 128:(j + 1) * 128, off:off + L], hh[:, 0:L])
                prev_len = L

    # All ACT funcs used (Exp, Ln, Identity) live in the single table
    # "natural_log_exp_and_others", but the table-load pass picks the FIRST
    # table containing each func. Empty out every other table (names and
    # positions preserved so runtime table ids stay valid) so a single
    # table load is emitted.
    import concourse.bacc as bacc_mod

    orig_tables = bacc_mod.get_activation_tables

    def _single_table(arch):
        tabs = orig_tables(arch)
        keep = "natural_log_exp_and_others"
        return {k: (v if k == keep else set()) for k, v in tabs.items()}

    bacc_mod.get_activation_tables = _single_table
    try:
        nc.compile()
    finally:
        bacc_mod.get_activation_tables = orig_tables
    return nc


def _get_nc():
    if "nc" not in _state:
        _state["nc"] = _build()
    return _state["nc"]


def _prep_inputs(x, h0, f_w, f_b, i_w, i_b, h_w, h_b):
    x = np.asarray(x, dtype=np.float32)
    h0 = np.asarray(h0, dtype=np.float32)
    import ml_dtypes

    bf16 = ml_dtypes.bfloat16
    xT = np.ascontiguousarray(x.transpose(0, 2, 1).astype(bf16))  # [B, D, T]
    hb = np.asarray(h_b, dtype=np.float32)
    shared = {}
    for p, w, bias in (("f", f_w, f_b), ("i", i_w, i_b), ("h", h_w, None)):
        w = np.asarray(w, dtype=np.float32)
        shared[f"w{p}"] = np.ascontiguousarray(w.T.astype(bf16))  # [D, H]
        if bias is not None:
            # f/i biases negated: kernel computes Exp(-pre + bias_ap)
            bias = -np.asarray(bias, dtype=np.float32)
            shared[f"b{p}"] = np.ascontiguousarray(bias.reshape(MH, 128).T)  # [128, MH]
    in_maps = []
    for b in range(B):
        m = dict(shared)
        m["xT"] = xT[b]
        # h_b is applied as a constant output shift (see kernel comment):
        # the device recurrence runs on h' = h - h_b.
        m["h0c"] = np.ascontiguousarray((h0[b, 0] - hb).reshape(MH, 128).T)
        in_maps.append(m)
    return in_maps


def kernel(x, h0, f_w, f_b, i_w, i_b, h_w, h_b, _trace=False):
    from concourse.bass_utils import run_bass_kernel_spmd

    nc = _get_nc()
    in_maps = _prep_inputs(x, h0, f_w, f_b, i_w, i_b, h_w, h_b)
    res = run_bass_kernel_spmd(nc, in_maps, core_ids=list(range(B)), trace=_trace)
    hb = np.asarray(h_b, dtype=np.float32)
    out = np.empty((B, T, H), dtype=np.float32)
    for b in range(B):
        out[b] = res.results[b]["hT"].astype(np.float32).T + hb
    if _trace:
        _state["last_results"] = res
    return out
